# revision 9
# baseline (speedup 1.0000x reference)
"""Distributed Trainium2 kernel for the ADD rotation loss.

Math: the reference computes mean_{b,n} || point[b,n] @ (R_pred[b] - R_gt[b]) ||
with R_pred/R_gt rotation matrices. Because both are rotations,

    || p @ (Rp - Rg) || = 2 * | p x qv |,

where qv is the vector part of the relative quaternion q_pred * conj(q_gt).
With {E1, E2} an orthogonal basis of the plane perpendicular to qv, each
scaled to length |qv|,

    | p x qv |^2 = (p . E1)^2 + (p . E2)^2.

The per-row coefficients E1/E2 (6 floats per batch row, ~0.006% of the
FLOPs) are computed on the host in float64, exactly mirroring the
reference euler->rotation convention; the device kernel is a pure
streaming pipeline over the 100 MB point tensor:

  per core (data-parallel over batch), per 128-row group:
    - DMA one fp8(e4m3) chunk holding the group's diag-stationary
      matrices followed by its points (planar x|y|z per row-half)
    - TensorE: v_j = p . E_j; the x/y planes go through one fp8
      DoubleRow matmul (2 k-subtiles), the z plane accumulates with a
      regular fp8 matmul
    - squares of the PSUM result split across DVE (custom NR op:
      (0 - (-1)*v)*v = v^2, single PSUM read) and ACT (Square)
    - Pool adds the two projection squares (paired across 2 groups)
    - ACT Sqrt(scale=4) with accum_out -> per-group partial sums
  Final tiny reduction (8 cores x 128 x slots) happens on the host.
"""

import sys

for _p in ("/opt/trn_rl_repo", "/root/.axon_site/_ro/trn_rl_repo"):
    if _p not in sys.path:
        sys.path.append(_p)

import numpy as np
import ml_dtypes

import concourse.bacc as bacc
import concourse.tile as tile
from concourse import mybir
from concourse.bass_utils import run_bass_kernel_spmd
from concourse.dve_ops import RECIPROCAL_APPROX_NR

NCORES = 8
B = 8192
N = 1024
BSH = B // NCORES          # batch rows per core
G = BSH // 128             # b-groups of 128 rows per core
H = N // 2                 # points per row-half
F32 = mybir.dt.float32
BF16 = mybir.dt.bfloat16
F8 = mybir.dt.float8e4
OP = mybir.AluOpType
AF = mybir.ActivationFunctionType
PM = mybir.MatmulPerfMode
E4M3 = ml_dtypes.float8_e4m3

W_G = 2 * 3 * 128          # fp8 stationary elems per row-group chunk (768)
PT_G = 2 * 3 * H           # fp8 point elems per row (3072)
C_G = W_G + PT_G           # chunk elems per (partition, group) (3840)
SQ_SPLIT = 1408            # PSUM cols squared on DVE; rest on ACT

_CACHE = {}


def build_bass():
    nc = bacc.Bacc("TRN2", target_bir_lowering=False, debug=False,
                   num_devices=NCORES)
    ptw = nc.declare_dram_parameter("ptw", [128, G * C_G], F8, isOutput=False)
    out = nc.declare_dram_parameter("out", [128, 8], F32, isOutput=True)

    def dv(a, b):
        return ptw[:, a:b]

    with tile.TileContext(nc) as tc:
        with (
            tc.tile_pool(name="const", bufs=1) as cp,
            tc.tile_pool(name="data", bufs=1) as dp,
            tc.tile_pool(name="psum", bufs=2, space="PSUM") as pp,
            tc.tile_pool(name="sq", bufs=2) as qp,
        ):
            # ---- persistent tiles ----
            acc = cp.tile([128, 8], F32, name="acc", tag="acc")
            wrm = cp.tile([128, 1], F32, name="wrm", tag="wrm")
            nc.vector.memset(wrm[:, :], 1.0)
            # warm the ACT sqrt table before any data arrives
            wrs = cp.tile([128, 1], F32, name="wrs", tag="wrs")
            nc.scalar.activation(out=wrs[:, :], in_=wrm[:, :], func=AF.Sqrt)
            # -1 constant for the DVE square trick: the NR custom op computes
            # (s0 - in0*in1)*in1, so in0 = -1, s0 = 0 gives in1^2 with a
            # single PSUM read (tensor_tensor mult would need two).
            mo = cp.tile([128, SQ_SPLIT], BF16, name="mo", tag="mo")
            nc.vector.memset(mo[:, :], -1.0)

            def dve_square(out_, in_, w):
                nc.vector._custom_dve(RECIPROCAL_APPROX_NR, out=out_,
                                      in0=mo[:, 0:w], in1=in_, s0=0.0)

            # ---- chunk tiles (one per DMA trigger => clean deps) ----
            # chunk layout per (partition, group): [W 768 | h0 1536 | h1 1536]
            C0a = dp.tile([128, W_G + 1536], F8, name="C0a", tag="C0a")
            C0b = dp.tile([128, 1536], F8, name="C0b", tag="C0b")
            C1 = dp.tile([128, C_G], F8, name="C1", tag="C1")
            C23 = dp.tile([128, 2, C_G], F8, name="C23", tag="C23")
            C45 = dp.tile([128, 2, C_G], F8, name="C45", tag="C45")
            C6 = dp.tile([128, C_G], F8, name="C6", tag="C6")
            C7a = dp.tile([128, W_G + 1536], F8, name="C7a", tag="C7a")
            C7b = dp.tile([128, 1536], F8, name="C7b", tag="C7b")

            # ---- DMA triggers (all Pool -> software-DGE spread over the
            # full DMA-engine fan-out; ordered by need time) ----
            nc.gpsimd.dma_start(out=C0a[:, :], in_=dv(0, W_G + 1536))
            nc.gpsimd.dma_start(out=C0b[:, :], in_=dv(W_G + 1536, C_G))
            nc.gpsimd.dma_start(out=C1[:, :], in_=dv(C_G, 2 * C_G))
            nc.gpsimd.dma_start(
                out=C23[:, :, :],
                in_=dv(2 * C_G, 4 * C_G).rearrange("p (g c) -> p g c", g=2))
            nc.gpsimd.dma_start(
                out=C45[:, :, :],
                in_=dv(4 * C_G, 6 * C_G).rearrange("p (g c) -> p g c", g=2))
            nc.gpsimd.dma_start(out=C6[:, :], in_=dv(6 * C_G, 7 * C_G))
            nc.gpsimd.dma_start(out=C7a[:, :],
                                in_=dv(7 * C_G, 7 * C_G + W_G + 1536))
            nc.gpsimd.dma_start(out=C7b[:, :],
                                in_=dv(7 * C_G + W_G + 1536, 8 * C_G))

            def views(g):
                # -> (wv [128,2(j),3,128], halves (h0, h1) each [128,3,H])
                if g == 0:
                    ca, cb = C0a, C0b
                elif g == 7:
                    ca, cb = C7a, C7b
                else:
                    c = {1: C1, 2: C23[:, 0], 3: C23[:, 1], 4: C45[:, 0],
                         5: C45[:, 1], 6: C6}[g]
                    ca, cb = c, None
                wv = ca[:, 0:W_G].rearrange("p (j c q) -> p j c q", j=2, c=3)
                ha = ca[:, W_G:W_G + 1536].rearrange("p (c n) -> p c n", c=3)
                if cb is None:
                    hb = ca[:, W_G + 1536:W_G + 3072].rearrange(
                        "p (c n) -> p c n", c=3)
                else:
                    hb = cb[:, :].rearrange("p (c n) -> p c n", c=3)
                return wv, (ha, hb)

            def emit_mm(pv, g):
                wv, hs = views(g)
                for h in (0, 1):
                    t = hs[h]
                    for j in (0, 1):
                        nc.tensor.matmul(out=pv[:, j, h, :],
                                         lhsT=wv[:, j, 0:2, :],
                                         rhs=t[:, 0:2, :],
                                         start=True, stop=False,
                                         perf_mode=PM.DoubleRow)
                        nc.tensor.matmul(out=pv[:, j, h, :],
                                         lhsT=wv[:, j, 2, :],
                                         rhs=t[:, 2, :],
                                         start=False, stop=True)

            # ---- main loop ----
            # groups 0..6: split squares (DVE custom-op / ACT), paired adds
            # on Pool, paired sqrts on ACT (slots 0..3). group 7 runs per
            # half-row-block to shorten the serial tail (slots 4,5).
            sqp = None
            for g in range(7):
                pv = pp.tile([128, 2, 2, H], F32, name="pv", tag="pv")
                emit_mm(pv, g)
                pvf = pv[:, :, :, :].rearrange("p j h n -> p (j h n)")
                if g % 2 == 0:
                    sqp = qp.tile([128, 2, 2048], BF16, name="sqp", tag="sqp")
                sq = sqp[:, g % 2, :]
                dve_square(sq[:, 0:SQ_SPLIT], pvf[:, 0:SQ_SPLIT], SQ_SPLIT)
                nc.scalar.activation(out=sq[:, SQ_SPLIT:2048],
                                     in_=pvf[:, SQ_SPLIT:2048],
                                     func=AF.Square)
                if g % 2 == 1:  # pairs (0,1),(2,3),(4,5) -> slots 0,1,2
                    s = g // 2
                    tot = qp.tile([128, 2, 1024], BF16, name="tot", tag="tot")
                    nc.gpsimd.tensor_tensor(out=tot[:, :, :],
                                            in0=sqp[:, :, 0:1024],
                                            in1=sqp[:, :, 1024:2048],
                                            op=OP.add)
                    dist = qp.tile([128, 2048], BF16, name="dist", tag="dist")
                    nc.scalar.activation(
                        out=dist[:, :],
                        in_=tot[:, :, :].rearrange("p a b -> p (a b)"),
                        func=AF.Sqrt, scale=4.0, accum_out=acc[:, s:s + 1])
                elif g == 6:    # slot 3
                    tot = qp.tile([128, 2, 1024], BF16, name="tot", tag="tot")
                    nc.gpsimd.tensor_tensor(out=tot[:, 0, :],
                                            in0=sqp[:, 0, 0:1024],
                                            in1=sqp[:, 0, 1024:2048],
                                            op=OP.add)
                    dist = qp.tile([128, 2048], BF16, name="dist", tag="dist")
                    nc.scalar.activation(
                        out=dist[:, 0:1024], in_=tot[:, 0, :],
                        func=AF.Sqrt, scale=4.0, accum_out=acc[:, 3:4])
                if g == 3:
                    nc.sync.dma_start(out=out[:, 0:2], in_=acc[:, 0:2])

            # group 7, per half: fine-grained tail
            pv = pp.tile([128, 2, 2, H], F32, name="pv", tag="pv")
            emit_mm(pv, 7)
            sq7 = qp.tile([128, 2, 2, H], BF16, name="sq7", tag="sq7")
            tot7 = qp.tile([128, 2, H], BF16, name="tot7", tag="tot7")
            dist7 = qp.tile([128, 2, H], BF16, name="dist7", tag="dist7")
            for h in (0, 1):
                dve_square(sq7[:, 0, h, :], pv[:, 0, h, :], H)
                nc.scalar.activation(out=sq7[:, 1, h, :],
                                     in_=pv[:, 1, h, :], func=AF.Square)
                nc.vector.tensor_tensor(out=tot7[:, h, :],
                                        in0=sq7[:, 0, h, :],
                                        in1=sq7[:, 1, h, :], op=OP.add)
                nc.scalar.activation(out=dist7[:, h, :], in_=tot7[:, h, :],
                                     func=AF.Sqrt, scale=4.0,
                                     accum_out=acc[:, 4 + h:5 + h])
            nc.sync.dma_start(out=out[:, 2:4], in_=acc[:, 2:4])
            nc.sync.dma_start(out=out[:, 4:6], in_=acc[:, 4:6])

    nc.finalize()
    return nc


# ---------------- host-side coefficient math ----------------

def _host_ew(pred, mode, gt):
    """E1/E2 per batch row, float64, mirroring the reference math."""
    p = pred.astype(np.float64)
    md = mode.astype(np.float64)
    m1, m2, m3, m4 = p[:, 0], p[:, 1], p[:, 2], p[:, 3]
    sgn = np.where(md > 0.5, 1.0, -1.0)
    e2 = sgn * np.arcsin(np.sqrt(m3 ** 2 / (m1 ** 2 + m2 ** 2 + m3 ** 2)))
    e3 = np.arctan2(m4, m3 / (np.sin(e2) + 1e-9))
    tmp = np.cos(e2) * np.cos(e3)
    e1 = np.arctan2(m2 / tmp, m1 / tmp)
    e3 = np.where(e3 > 0, e3, e3 + 2 * np.pi)
    ep = np.stack([e1, e2, e3], -1)
    eg = gt.astype(np.float64)

    def quat_xyz(e):
        # q = qx(a) * qy(b) * qz(c) for R = Rx(a) Ry(b) Rz(c)
        a, b, c = e[:, 0] / 2, e[:, 1] / 2, e[:, 2] / 2
        ca, sa = np.cos(a), np.sin(a)
        cb, sb = np.cos(b), np.sin(b)
        cc, sc = np.cos(c), np.sin(c)
        w = ca * cb * cc - sa * sb * sc
        x = sa * cb * cc + ca * sb * sc
        y = ca * sb * cc - sa * cb * sc
        z = ca * cb * sc + sa * sb * cc
        return w, np.stack([x, y, z], -1)

    wp, vp = quat_xyz(ep)
    wg, vg = quat_xyz(eg)
    qv = wg[:, None] * vp - wp[:, None] * vg - np.cross(vp, vg)

    qx, qy, qz = qv[:, 0], qv[:, 1], qv[:, 2]
    s = qy ** 2 + qz ** 2
    n = np.sqrt(s + qx ** 2)
    r = 1.0 / np.sqrt(s + 1e-250)
    t1 = n * r
    zero = np.zeros_like(qx)
    E1 = np.stack([zero, qz * t1, -qy * t1], -1)
    E2 = np.stack([-s * r, qx * qy * r, qx * qz * r], -1)
    return np.stack([E1, E2], 1)   # [B, 2, 3]


def _pack_inputs(pred, mode, gt, point):
    ew = _host_ew(np.asarray(pred), np.asarray(mode), np.asarray(gt))
    ewq = ew.astype(np.float32).astype(E4M3)           # [B, 2, 3]
    ptq = np.asarray(point, dtype=np.float32).astype(E4M3)  # [B, N, 3]

    in_maps = []
    idx = np.arange(128)
    for c in range(NCORES):
        sl = slice(c * BSH, (c + 1) * BSH)
        # row b_local = p*G + g
        ewc = ewq[sl].reshape(128, G, 2, 3)
        wtc = np.zeros((128, G, 2, 3, 128), dtype=E4M3)
        wtc[idx, :, :, :, idx] = ewc
        ptc = (ptq[sl].reshape(128, G, 2, H, 3)
               .transpose(0, 1, 2, 4, 3))              # [p, g, h, c, n]
        chunk = np.concatenate(
            [wtc.reshape(128, G, W_G), ptc.reshape(128, G, PT_G)], axis=2)
        in_maps.append({"ptw": np.ascontiguousarray(chunk)
                        .reshape(128, G * C_G)})
    return in_maps


def _get_nc():
    if "nc" not in _CACHE:
        _CACHE["nc"] = build_bass()
    return _CACHE["nc"]


def kernel(pred, mode, gt, point, **run_kwargs):
    nc = _get_nc()
    in_maps = _pack_inputs(pred, mode, gt, point)
    res = run_bass_kernel_spmd(nc, in_maps, core_ids=list(range(NCORES)),
                               **run_kwargs)
    total = sum(float(r["out"][:, 0:6].astype(np.float64).sum())
                for r in res.results)
    result = np.float32(total / (B * N))
    if run_kwargs:
        return result, res
    return result


# revision 17
# speedup vs baseline: 1.2082x; 1.2082x over previous
"""Distributed Trainium2 kernel for the ADD rotation loss.

Math: the reference computes mean_{b,n} || point[b,n] @ (R_pred[b] - R_gt[b]) ||
with R_pred/R_gt rotation matrices. Because both are rotations,

    || p @ (Rp - Rg) || = 2 * | p x qv |,

where qv is the vector part of the relative quaternion q_pred * conj(q_gt).
With {E1, E2} an orthogonal basis of the plane perpendicular to qv, each
scaled to length |qv|,

    | p x qv |^2 = (p . E1)^2 + (p . E2)^2.

The per-row coefficients E1/E2 (6 floats per batch row, ~0.006% of the
FLOPs) are computed on the host in float64, exactly mirroring the
reference euler->rotation convention; the device kernel is a pure
streaming pipeline over the 100 MB point tensor:

  per core (data-parallel over batch), per 128-row group:
    - DMA one fp8(e4m3) chunk holding the group's diag-stationary
      matrices followed by its points (planar x|y|z per row-half)
    - TensorE: v_j = p . E_j; the x/y planes go through one fp8
      DoubleRow matmul (2 k-subtiles), the z plane accumulates with a
      regular fp8 matmul
    - squares of the PSUM result split across DVE (custom NR op:
      (0 - (-1)*v)*v = v^2, single PSUM read) and ACT (Square)
    - Pool adds the two projection squares (paired across 2 groups)
    - ACT Sqrt(scale=4) with accum_out -> per-group partial sums
  Final tiny reduction (8 cores x 128 x slots) happens on the host.
"""

import sys

for _p in ("/opt/trn_rl_repo", "/root/.axon_site/_ro/trn_rl_repo"):
    if _p not in sys.path:
        sys.path.append(_p)

import numpy as np
import ml_dtypes

import concourse.bacc as bacc
import concourse.tile as tile
from concourse import mybir
from concourse.bass_utils import run_bass_kernel_spmd
from concourse.dve_ops import RECIPROCAL_APPROX_NR

NCORES = 8
B = 8192
N = 1024
BSH = B // NCORES          # batch rows per core
G = BSH // 128             # b-groups of 128 rows per core
H = N // 2                 # points per row-half
F32 = mybir.dt.float32
BF16 = mybir.dt.bfloat16
F8 = mybir.dt.float8e4
OP = mybir.AluOpType
AF = mybir.ActivationFunctionType
PM = mybir.MatmulPerfMode
E4M3 = ml_dtypes.float8_e4m3

W_G = 2 * 3 * 128          # fp8 stationary elems per row-group chunk (768)
PT_G = 2 * 3 * H           # fp8 point elems per row (3072)
C_G = W_G + PT_G           # chunk elems per (partition, group) (3840)
SQ_SPLIT = 1216            # PSUM cols squared on DVE; rest on ACT

_CACHE = {}


def build_bass():
    nc = bacc.Bacc("TRN2", target_bir_lowering=False, debug=False,
                   num_devices=NCORES)
    ptw = nc.declare_dram_parameter("ptw", [128, G * C_G], F8, isOutput=False)
    out = nc.declare_dram_parameter("out", [128, 12], F32, isOutput=True)

    def dv(a, b):
        return ptw[:, a:b]

    with tile.TileContext(nc) as tc:
        with (
            tc.tile_pool(name="const", bufs=1) as cp,
            tc.tile_pool(name="data", bufs=1) as dp,
            tc.tile_pool(name="psum", bufs=2, space="PSUM") as pp,
            tc.tile_pool(name="sq", bufs=2) as qp,
        ):
            # ---- persistent tiles ----
            acc = cp.tile([128, 12], F32, name="acc", tag="acc")
            wrm = cp.tile([128, 1], F32, name="wrm", tag="wrm")
            nc.vector.memset(wrm[:, :], 1.0)
            # warm the ACT sqrt table before any data arrives
            wrs = cp.tile([128, 1], F32, name="wrs", tag="wrs")
            nc.scalar.activation(out=wrs[:, :], in_=wrm[:, :], func=AF.Sqrt)
            # -1 constant for the DVE square trick: the NR custom op computes
            # (s0 - in0*in1)*in1, so in0 = -1, s0 = 0 gives in1^2 with a
            # single PSUM read (tensor_tensor mult would need two).
            mo = cp.tile([128, SQ_SPLIT], BF16, name="mo", tag="mo")
            nc.vector.memset(mo[:, :], -1.0)

            def dve_square(out_, in_, w, shape=None):
                m = mo[:, 0:w]
                if shape is not None:
                    m = m.rearrange("p (a b) -> p a b", a=shape[0])
                nc.vector._custom_dve(RECIPROCAL_APPROX_NR, out=out_,
                                      in0=m, in1=in_, s0=0.0)

            # ---- chunk tiles (one per DMA trigger => clean deps) ----
            # chunk layout per (partition, group): [W 768 | h0 1536 | h1 1536]
            C0a = dp.tile([128, W_G + 1536], F8, name="C0a", tag="C0a")
            C0b = dp.tile([128, 1536], F8, name="C0b", tag="C0b")
            C1 = dp.tile([128, C_G], F8, name="C1", tag="C1")
            C23 = dp.tile([128, 2, C_G], F8, name="C23", tag="C23")
            C45 = dp.tile([128, 2, C_G], F8, name="C45", tag="C45")
            C6 = dp.tile([128, C_G], F8, name="C6", tag="C6")
            C7a = dp.tile([128, W_G + 1536], F8, name="C7a", tag="C7a")
            C7b = dp.tile([128, 1536], F8, name="C7b", tag="C7b")

            # ---- DMA triggers (all Pool -> software-DGE spread over the
            # full DMA-engine fan-out; ordered by need time) ----
            nc.gpsimd.dma_start(out=C0a[:, :], in_=dv(0, W_G + 1536))
            nc.gpsimd.dma_start(out=C0b[:, :], in_=dv(W_G + 1536, C_G))
            nc.gpsimd.dma_start(out=C1[:, :], in_=dv(C_G, 2 * C_G))
            nc.gpsimd.dma_start(
                out=C23[:, :, :],
                in_=dv(2 * C_G, 4 * C_G).rearrange("p (g c) -> p g c", g=2))
            nc.gpsimd.dma_start(
                out=C45[:, :, :],
                in_=dv(4 * C_G, 6 * C_G).rearrange("p (g c) -> p g c", g=2))
            nc.gpsimd.dma_start(out=C6[:, :], in_=dv(6 * C_G, 7 * C_G))
            nc.gpsimd.dma_start(out=C7a[:, :],
                                in_=dv(7 * C_G, 7 * C_G + W_G + 1536))
            nc.gpsimd.dma_start(out=C7b[:, :],
                                in_=dv(7 * C_G + W_G + 1536, 8 * C_G))

            def views(g):
                # -> (wv [128,2(j),3,128], halves (h0, h1) each [128,3,H])
                if g == 0:
                    ca, cb = C0a, C0b
                elif g == 7:
                    ca, cb = C7a, C7b
                else:
                    c = {1: C1, 2: C23[:, 0], 3: C23[:, 1], 4: C45[:, 0],
                         5: C45[:, 1], 6: C6}[g]
                    ca, cb = c, None
                wv = ca[:, 0:W_G].rearrange("p (j c q) -> p j c q", j=2, c=3)
                ha = ca[:, W_G:W_G + 1536].rearrange("p (c n) -> p c n", c=3)
                if cb is None:
                    hb = ca[:, W_G + 1536:W_G + 3072].rearrange(
                        "p (c n) -> p c n", c=3)
                else:
                    hb = cb[:, :].rearrange("p (c n) -> p c n", c=3)
                return wv, (ha, hb)

            def emit_mm(pv, g):
                wv, hs = views(g)
                for h in (0, 1):
                    t = hs[h]
                    for j in (0, 1):
                        nc.tensor.matmul(out=pv[:, j, h, :],
                                         lhsT=wv[:, j, 0:2, :],
                                         rhs=t[:, 0:2, :],
                                         start=True, stop=False,
                                         perf_mode=PM.DoubleRow)
                        nc.tensor.matmul(out=pv[:, j, h, :],
                                         lhsT=wv[:, j, 2, :],
                                         rhs=t[:, 2, :],
                                         start=False, stop=True)

            # ---- main loop ----
            # groups 0..6: squares split DVE/ACT per group, add on DVE,
            # per-group sqrt on ACT. The sqrt for group g is EMITTED after
            # the squares of group g+1 so ACT's in-order stream never
            # stalls waiting on DVE's add. group 7 runs per half-row-block
            # with the halves split across engines to shorten the tail.
            pend = None           # (tot_view, slot) awaiting sqrt emission

            def emit_sqrt(tot_view, slot):
                dist = qp.tile([128, 1024], BF16, name="dist", tag="dist")
                nc.scalar.activation(out=dist[:, :], in_=tot_view,
                                     func=AF.Sqrt, scale=4.0,
                                     accum_out=acc[:, slot:slot + 1])

            for g in range(7):
                pv = pp.tile([128, 2, 2, H], F32, name="pv", tag="pv")
                emit_mm(pv, g)
                pvf = pv[:, :, :, :].rearrange("p j h n -> p (j h n)")
                sq = qp.tile([128, 2048], BF16, name="sq", tag="sq")
                dve_square(sq[:, 0:SQ_SPLIT], pvf[:, 0:SQ_SPLIT], SQ_SPLIT)
                nc.scalar.activation(out=sq[:, SQ_SPLIT:2048],
                                     in_=pvf[:, SQ_SPLIT:2048],
                                     func=AF.Square)
                tot = qp.tile([128, 1024], BF16, name="tot", tag="tot")
                nc.vector.tensor_tensor(out=tot[:, :], in0=sq[:, 0:1024],
                                        in1=sq[:, 1024:2048], op=OP.add)
                if pend is not None:
                    emit_sqrt(*pend)
                pend = (tot[:, :], g)
                if g == 4:
                    nc.sync.dma_start(out=out[:, 0:4], in_=acc[:, 0:4])

            # group 7, halves split across engines:
            #   h0 squares on DVE (both j), h1 squares on ACT (both j)
            pv = pp.tile([128, 2, 2, H], F32, name="pv", tag="pv")
            emit_mm(pv, 7)
            sq7 = qp.tile([128, 2, 2, H], BF16, name="sq7", tag="sq7")
            tot7 = qp.tile([128, 2, H], BF16, name="tot7", tag="tot7")
            dist7 = qp.tile([128, 2, H], BF16, name="dist7", tag="dist7")
            dve_square(sq7[:, :, 0, :], pv[:, :, 0, :], 1024, shape=(2, H))
            nc.scalar.activation(out=sq7[:, :, 1, :], in_=pv[:, :, 1, :],
                                 func=AF.Square)
            if pend is not None:   # sqrt(g6) overlaps g7 squares
                emit_sqrt(*pend)
            for h in (0, 1):
                nc.vector.tensor_tensor(out=tot7[:, h, :],
                                        in0=sq7[:, 0, h, :],
                                        in1=sq7[:, 1, h, :], op=OP.add)
                nc.scalar.activation(out=dist7[:, h, :], in_=tot7[:, h, :],
                                     func=AF.Sqrt, scale=4.0,
                                     accum_out=acc[:, 7 + h:8 + h])
            nc.sync.dma_start(out=out[:, 4:7], in_=acc[:, 4:7])
            nc.sync.dma_start(out=out[:, 7:9], in_=acc[:, 7:9])

    nc.finalize()
    return nc


# ---------------- host-side coefficient math ----------------

def _host_ew(pred, mode, gt):
    """E1/E2 per batch row, float64, mirroring the reference math."""
    p = pred.astype(np.float64)
    md = mode.astype(np.float64)
    m1, m2, m3, m4 = p[:, 0], p[:, 1], p[:, 2], p[:, 3]
    sgn = np.where(md > 0.5, 1.0, -1.0)
    e2 = sgn * np.arcsin(np.sqrt(m3 ** 2 / (m1 ** 2 + m2 ** 2 + m3 ** 2)))
    e3 = np.arctan2(m4, m3 / (np.sin(e2) + 1e-9))
    tmp = np.cos(e2) * np.cos(e3)
    e1 = np.arctan2(m2 / tmp, m1 / tmp)
    e3 = np.where(e3 > 0, e3, e3 + 2 * np.pi)
    ep = np.stack([e1, e2, e3], -1)
    eg = gt.astype(np.float64)

    def quat_xyz(e):
        # q = qx(a) * qy(b) * qz(c) for R = Rx(a) Ry(b) Rz(c)
        a, b, c = e[:, 0] / 2, e[:, 1] / 2, e[:, 2] / 2
        ca, sa = np.cos(a), np.sin(a)
        cb, sb = np.cos(b), np.sin(b)
        cc, sc = np.cos(c), np.sin(c)
        w = ca * cb * cc - sa * sb * sc
        x = sa * cb * cc + ca * sb * sc
        y = ca * sb * cc - sa * cb * sc
        z = ca * cb * sc + sa * sb * cc
        return w, np.stack([x, y, z], -1)

    wp, vp = quat_xyz(ep)
    wg, vg = quat_xyz(eg)
    qv = wg[:, None] * vp - wp[:, None] * vg - np.cross(vp, vg)

    qx, qy, qz = qv[:, 0], qv[:, 1], qv[:, 2]
    s = qy ** 2 + qz ** 2
    n = np.sqrt(s + qx ** 2)
    r = 1.0 / np.sqrt(s + 1e-250)
    t1 = n * r
    zero = np.zeros_like(qx)
    E1 = np.stack([zero, qz * t1, -qy * t1], -1)
    E2 = np.stack([-s * r, qx * qy * r, qx * qz * r], -1)
    return np.stack([E1, E2], 1)   # [B, 2, 3]


def _pack_inputs(pred, mode, gt, point):
    ew = _host_ew(np.asarray(pred), np.asarray(mode), np.asarray(gt))
    ewq = ew.astype(np.float32).astype(E4M3)           # [B, 2, 3]
    ptq = np.asarray(point, dtype=np.float32).astype(E4M3)  # [B, N, 3]

    in_maps = []
    idx = np.arange(128)
    for c in range(NCORES):
        sl = slice(c * BSH, (c + 1) * BSH)
        # row b_local = p*G + g
        ewc = ewq[sl].reshape(128, G, 2, 3)
        wtc = np.zeros((128, G, 2, 3, 128), dtype=E4M3)
        wtc[idx, :, :, :, idx] = ewc
        ptc = (ptq[sl].reshape(128, G, 2, H, 3)
               .transpose(0, 1, 2, 4, 3))              # [p, g, h, c, n]
        chunk = np.concatenate(
            [wtc.reshape(128, G, W_G), ptc.reshape(128, G, PT_G)], axis=2)
        in_maps.append({"ptw": np.ascontiguousarray(chunk)
                        .reshape(128, G * C_G)})
    return in_maps


def _get_nc():
    if "nc" not in _CACHE:
        _CACHE["nc"] = build_bass()
    return _CACHE["nc"]


def kernel(pred, mode, gt, point, **run_kwargs):
    nc = _get_nc()
    in_maps = _pack_inputs(pred, mode, gt, point)
    res = run_bass_kernel_spmd(nc, in_maps, core_ids=list(range(NCORES)),
                               **run_kwargs)
    total = sum(float(r["out"][:, 0:9].astype(np.float64).sum())
                for r in res.results)
    result = np.float32(total / (B * N))
    if run_kwargs:
        return result, res
    return result
